# revision 4
# baseline (speedup 1.0000x reference)
"""GQA attention (RoPE, causal) + output projection for Trainium2, 8 NeuronCores.

Problem: B=2, T=2048, HID=2048, NH=16 Q-heads, NKV=4 KV-heads, HD=128.
Sharding: tensor-parallel over the 4 KV-head groups (4 Q heads + 1 KV head per
group) x data-parallel over batch (2). Core c handles batch c//4, group c%4.
Each core computes its group's partial output y_g = A_g @ Wo[rows_g]; the
host unshards by summing the 4 row-parallel partials per batch.

Per-core device pipeline (all matmuls bf16, accumulation f32 in PSUM):
  1. x [T,HID] f32 -> cast bf16 -> DMA-transpose to xT [h,T]
  2. QKV projections -> natural layout [t, cols]; RoPE applied with free-dim
     rotate-half; Q/K DMA-transposed to [d, T]; V kept natural with an
     appended ones-column (gives softmax row-sums for free).
  3. Scores computed transposed: ST[kv,q] = K @ Q^T (1/sqrt(HD) folded into
     the Q rope tables); exp on ScalarE (no max subtraction needed: scores
     are ~N(0,1) so exp never overflows); causal mask = multiplicative bf16
     mask on diagonal tiles; A~ = expST^T @ [V|1] accumulated over kv chunks;
     normalize by reciprocal of the ones-column.
  4. A tiles DMA-transposed to aT [d, T]; y = A @ Wo via lhsT=aT.
"""

import numpy as np
import ml_dtypes

import concourse.bass as bass
import concourse.mybir as mybir
import concourse.tile as tile
from concourse import bacc
from concourse.bass_utils import run_bass_kernel_spmd

B, T, HID = 2, 2048, 2048
NH, NKV = 16, 4
HD = 128
GROUPS = NH // NKV      # 4 q-heads per kv head
NQ = GROUPS             # q heads per core
QW = NQ * HD            # 512 q cols per core
P = 128
TB = T // P             # 16 t-blocks
HC = HID // P           # 16 hid chunks
QS = T // 512           # 4 q supertiles
KVC = T // P            # 16 kv chunks
ROPE_BASE = 10000.0

F32 = mybir.dt.float32
BF16 = mybir.dt.bfloat16


def build_nc():
    nc = bacc.Bacc("TRN2", target_bir_lowering=False, debug=False,
                   enable_asserts=False, num_devices=8)

    x_d = nc.dram_tensor("x", [T, HID], F32, kind="ExternalInput")
    wq_d = nc.dram_tensor("wq", [HID, QW], F32, kind="ExternalInput")
    wkv_d = nc.dram_tensor("wkv", [HID, 2 * HD], F32, kind="ExternalInput")
    wo_d = nc.dram_tensor("wo", [QW, HID], F32, kind="ExternalInput")
    cosq_d = nc.dram_tensor("cosq", [T, HD], BF16, kind="ExternalInput")
    sinq_d = nc.dram_tensor("sinq", [T, HD], BF16, kind="ExternalInput")
    cosk_d = nc.dram_tensor("cosk", [T, HD], BF16, kind="ExternalInput")
    sink_d = nc.dram_tensor("sink", [T, HD], BF16, kind="ExternalInput")
    masks_d = nc.dram_tensor("masks", [4, P, 512], BF16, kind="ExternalInput")
    y_d = nc.dram_tensor("y", [T, HID], F32, kind="ExternalOutput")

    with tile.TileContext(nc) as tc:
        with (
            tc.tile_pool(name="persist", bufs=1) as persist,
            tc.tile_pool(name="stage", bufs=3) as stage,
        ):
            # ---- persistent SBUF buffers ----
            qT = persist.tile([P, NQ, T], BF16)           # 2 MB  (d, h, t)
            kT = persist.tile([P, T], BF16)               # 0.5 MB(d, t)
            vko = persist.tile([P, KVC, HD + 1], BF16)    # 0.5 MB(t, kvc, d|1)
            aT = persist.tile([P, NQ, T], BF16)           # 2 MB  (d, h, t)
            wq_s = persist.tile([P, HC, QW], BF16)        # 2 MB
            wkv_s = persist.tile([P, HC, 2 * HD], BF16)   # 1 MB
            wo_s = persist.tile([P, NQ, HID], BF16)       # 2 MB
            cosq_s = persist.tile([P, TB, HD], BF16)      # (t, tb, d)
            sinq_s = persist.tile([P, TB, HD], BF16)
            cosk_s = persist.tile([P, TB, HD], BF16)
            sink_s = persist.tile([P, TB, HD], BF16)
            masks_s = persist.tile([P, 4, 512], BF16)

            # ---- constants / weights load + cast ----
            nc.sync.dma_start(masks_s[:], masks_d.rearrange("o p q -> p o q"))
            nc.sync.dma_start(cosq_s[:], cosq_d.rearrange("(tb p) d -> p tb d", p=P))
            nc.sync.dma_start(sinq_s[:], sinq_d.rearrange("(tb p) d -> p tb d", p=P))
            nc.sync.dma_start(cosk_s[:], cosk_d.rearrange("(tb p) d -> p tb d", p=P))
            nc.sync.dma_start(sink_s[:], sink_d.rearrange("(tb p) d -> p tb d", p=P))
            nc.vector.memset(vko[:, :, HD:HD + 1], 1.0)

            for hc in range(HC):
                wtmp = stage.tile([P, QW], F32, tag="wtmp")
                nc.sync.dma_start(wtmp[:], wq_d[hc * P:(hc + 1) * P, :])
                nc.vector.tensor_copy(wq_s[:, hc, :], wtmp[:])
                wtmp2 = stage.tile([P, 2 * HD], F32, tag="wtmp2")
                nc.sync.dma_start(wtmp2[:], wkv_d[hc * P:(hc + 1) * P, :])
                nc.vector.tensor_copy(wkv_s[:, hc, :], wtmp2[:])
            for cc in range(NQ):
                wtmp3 = stage.tile([P, HID], F32, tag="wtmp3", bufs=2)
                nc.sync.dma_start(wtmp3[:], wo_d[cc * P:(cc + 1) * P, :])
                nc.vector.tensor_copy(wo_s[:, cc, :], wtmp3[:])

            # ---- stage 1+2: x load/cast/transpose fused with projections ----
            with tc.tile_pool(name="psA", bufs=2, space="PSUM") as psA:
                for tb in range(TB):
                    xf = stage.tile([P, HID], F32, tag="xf", bufs=2)
                    nc.sync.dma_start(xf[:], x_d[tb * P:(tb + 1) * P, :])
                    xb = stage.tile([P, HID], BF16, tag="xb", bufs=2)
                    nc.vector.tensor_copy(xb[:], xf[:])
                    xTt = stage.tile([P, HC, P], BF16, tag="xTt", bufs=2)
                    for hc in range(HC):
                        nc.sync.dma_start_transpose(
                            xTt[:, hc, :], xb[:, hc * P:(hc + 1) * P])
                    q_ps = psA.tile([P, QW], F32, tag="qps")
                    kv_ps = psA.tile([P, 2 * HD], F32, tag="kvps")
                    for hc in range(HC):
                        xt_blk = xTt[:, hc, :]
                        nc.tensor.matmul(q_ps[:], xt_blk, wq_s[:, hc, :],
                                         start=(hc == 0), stop=(hc == HC - 1))
                        nc.tensor.matmul(kv_ps[:], xt_blk, wkv_s[:, hc, :],
                                         start=(hc == 0), stop=(hc == HC - 1))
                    # rope Q: view [P, NQ, HD]
                    qv = q_ps.rearrange("p (h d) -> p h d", h=NQ)
                    rot = stage.tile([P, NQ, HD], F32, tag="rot")
                    nc.vector.tensor_copy(rot[:, :, 0:64], qv[:, :, 64:128])
                    nc.vector.tensor_copy(rot[:, :, 64:128], qv[:, :, 0:64])
                    qc = stage.tile([P, NQ, HD], F32, tag="qc")
                    cq = cosq_s[:, tb, None, :].to_broadcast([P, NQ, HD])
                    sq = sinq_s[:, tb, None, :].to_broadcast([P, NQ, HD])
                    nc.vector.tensor_mul(qc[:], qv, cq)
                    nc.vector.tensor_mul(rot[:], rot[:], sq)
                    qout = stage.tile([P, NQ, HD], BF16, tag="qout")
                    nc.vector.tensor_add(qout[:], qc[:], rot[:])
                    for h in range(NQ):
                        nc.sync.dma_start_transpose(
                            qT[:, h, tb * P:(tb + 1) * P], qout[:, h, :])
                    # rope K
                    kv = kv_ps[:, 0:HD]
                    rotk = stage.tile([P, HD], F32, tag="rotk")
                    nc.vector.tensor_copy(rotk[:, 0:64], kv[:, 64:128])
                    nc.vector.tensor_copy(rotk[:, 64:128], kv[:, 0:64])
                    kc = stage.tile([P, HD], F32, tag="kc")
                    nc.vector.tensor_mul(kc[:], kv, cosk_s[:, tb, :])
                    nc.vector.tensor_mul(rotk[:], rotk[:], sink_s[:, tb, :])
                    kout = stage.tile([P, HD], BF16, tag="kout")
                    nc.vector.tensor_add(kout[:], kc[:], rotk[:])
                    nc.sync.dma_start_transpose(kT[:, tb * P:(tb + 1) * P], kout[:])
                    # V cast (natural layout; ones col already set)
                    nc.vector.tensor_copy(vko[:, tb, 0:HD], kv_ps[:, HD:2 * HD])

            # ---- stage 3: attention ----
            with (
                tc.tile_pool(name="psS", bufs=2, space="PSUM") as psS,
                tc.tile_pool(name="psAv", bufs=1, space="PSUM") as psAv,
            ):
                for h in range(NQ):
                    for qs in range(QS):
                        nkv = (qs + 1) * 4
                        av = psAv.tile([P, NQ, 512], F32, tag="av")
                        for kvc in range(nkv):
                            st_ps = psS.tile([P, 512], F32, tag="st")
                            nc.tensor.matmul(
                                st_ps[:], kT[:, kvc * P:(kvc + 1) * P],
                                qT[:, h, qs * 512:(qs + 1) * 512],
                                start=True, stop=True)
                            pst = stage.tile([P, 512], BF16, tag="pst")
                            nc.scalar.activation(
                                pst[:], st_ps[:], mybir.ActivationFunctionType.Exp)
                            o = kvc - 4 * qs
                            if o >= 0:
                                nc.vector.tensor_mul(pst[:], pst[:], masks_s[:, o, :])
                            for qb in range(4):
                                nc.tensor.matmul(
                                    av[:, qb, 0:HD + 1],
                                    pst[:, qb * P:(qb + 1) * P],
                                    vko[:, kvc, :],
                                    start=(kvc == 0), stop=(kvc == nkv - 1))
                        for qb in range(4):
                            r = stage.tile([P, 1], F32, tag="r")
                            nc.vector.reciprocal(r[:], av[:, qb, HD:HD + 1])
                            a_sb = stage.tile([P, HD], BF16, tag="asb")
                            nc.vector.tensor_scalar_mul(a_sb[:], av[:, qb, 0:HD], r[:])
                            tq = qs * 4 + qb
                            nc.sync.dma_start_transpose(
                                aT[:, h, tq * P:(tq + 1) * P], a_sb[:])

            # ---- stage 4: output projection ----
            with tc.tile_pool(name="psY", bufs=2, space="PSUM") as psY:
                for tb in range(TB):
                    y_ps = psY.tile([P, NQ, 512], F32, tag="yps")
                    for cc in range(NQ):
                        at_blk = aT[:, cc, tb * P:(tb + 1) * P]
                        for ns in range(4):
                            nc.tensor.matmul(
                                y_ps[:, ns, :], at_blk,
                                wo_s[:, cc, ns * 512:(ns + 1) * 512],
                                start=(cc == 0), stop=(cc == NQ - 1))
                    for ns in range(4):
                        y_sb = stage.tile([P, 512], F32, tag="ysb")
                        nc.scalar.copy(y_sb[:], y_ps[:, ns, :])
                        nc.sync.dma_start(
                            y_d[tb * P:(tb + 1) * P, ns * 512:(ns + 1) * 512],
                            y_sb[:])

    nc.compile()
    return nc


def make_tables():
    inv_freq = 1.0 / (ROPE_BASE ** (np.arange(0, HD, 2, dtype=np.float64) / HD))
    t = np.arange(T, dtype=np.float64)
    freqs = np.outer(t, inv_freq)                       # [T, 64]
    emb = np.concatenate([freqs, freqs], axis=-1)       # [T, 128]
    cos = np.cos(emb)
    sin = np.sin(emb)
    sin_signed = sin.copy()
    sin_signed[:, :64] = -sin_signed[:, :64]
    scale = 1.0 / np.sqrt(HD)
    cosq = (cos * scale).astype(ml_dtypes.bfloat16)
    sinq = (sin_signed * scale).astype(ml_dtypes.bfloat16)
    cosk = cos.astype(ml_dtypes.bfloat16)
    sink = sin_signed.astype(ml_dtypes.bfloat16)
    return cosq, sinq, cosk, sink


def make_masks():
    # masks[o][i, j] = 1 if (o*128 + i) <= j else 0   (ST tile [kv=128, q=512])
    masks = np.zeros((4, P, 512), dtype=ml_dtypes.bfloat16)
    j = np.arange(512)[None, :]
    i = np.arange(P)[:, None]
    for o in range(4):
        masks[o] = ((o * P + i) <= j).astype(ml_dtypes.bfloat16)
    return masks


def make_in_maps(x, Wq, Wk, Wv, Wo):
    cosq, sinq, cosk, sink = make_tables()
    masks = make_masks()
    in_maps = []
    for c in range(8):
        b, g = c // 4, c % 4
        in_maps.append({
            "x": np.ascontiguousarray(x[b]),
            "wq": np.ascontiguousarray(Wq[:, g * QW:(g + 1) * QW]),
            "wkv": np.ascontiguousarray(
                np.concatenate([Wk[:, g * HD:(g + 1) * HD],
                                Wv[:, g * HD:(g + 1) * HD]], axis=1)),
            "wo": np.ascontiguousarray(Wo[g * QW:(g + 1) * QW, :]),
            "cosq": cosq, "sinq": sinq, "cosk": cosk, "sink": sink,
            "masks": masks,
        })
    return in_maps


_NC_CACHE = None


def kernel(x, Wq, Wk, Wv, Wo, _trace=False, _tmpdir=None):
    global _NC_CACHE
    x = np.asarray(x, dtype=np.float32)
    Wq = np.asarray(Wq, dtype=np.float32)
    Wk = np.asarray(Wk, dtype=np.float32)
    Wv = np.asarray(Wv, dtype=np.float32)
    Wo = np.asarray(Wo, dtype=np.float32)

    if _NC_CACHE is None:
        _NC_CACHE = build_nc()
    nc = _NC_CACHE

    in_maps = make_in_maps(x, Wq, Wk, Wv, Wo)
    res = run_bass_kernel_spmd(nc, in_maps, core_ids=list(range(8)),
                               trace=_trace, tmpdir=_tmpdir)
    out = np.zeros((B, T, HID), dtype=np.float32)
    for c in range(8):
        out[c // 4] += res.results[c]["y"]
    if _trace:
        return out, res
    return out
